# revision 44
# baseline (speedup 1.0000x reference)
"""Trainium2 Bass kernel for BracketGNN (3-layer GCN + mean-pool + MLP head).

Strategy (8 NeuronCores, SPMD):
  - Nodes sharded contiguously across cores (6250 dst nodes / core); each core
    owns the incident edges of its dst nodes.
  - Per layer the full node table T_l[n] = dinv_n * (h_l @ W'_l)  (bf16,
    [N,128] rows in DRAM) is AllGathered; each core edge-gathers rows of its
    in-edges with the MoE dma_gather primitive (int16 indices -> the table is
    addressed through two base views, src<32768 and src>=32768).
  - The gathers are issued round-robin over the 4 SWDGE queues so descriptor
    generation runs on all four Q7 core pairs concurrently (the dominant cost
    of this kernel is SWDGE descriptor generation at ~7.4 ns/row/pair); small
    GCH (6 chunks/call) + deep msg/seg pools keep all four queues fed.
  - Aggregation = per-128-edge-chunk matmul:  psum[feat, dst] +=
    msg_chunk[128e x 128f].T @ seg_chunk[128e x 128dst]  where seg is the
    one-hot (edge -> local dst) matrix built on-device by DVE compare against
    a host-provided dst-column table.  Chunk schedule is uniform across cores
    (SPMD); per-core variation lives in the seg/idx data.
  - Self-loop messages never go through the gather: the aggregation psum is
    seeded with usave * dinv_d (usave = the block's own pre-dinv transform,
    saved from the previous layer's epilogue), which after the epilogue's
    * dinv_d equals the dinv_d^2 * (h @ W') self-loop term.
  - Epilogue per 128-dst block: x dinv_d, +B (folded BN bias), ReLU, next-layer
    weight transform (BN scale folded into W on host), x dinv_n, PE-transpose
    back to node-major rows, DMA into the next AllGather input.
  - After layer 3: per-graph mean pool via one-hot graph matmul (one-hot
    streamed from DRAM), bf16 AllReduce of the [128f x 256g] partial sums,
    then the (replicated) fp32 classifier head.
"""

import os
import sys
import numpy as np

sys.path.insert(0, "/opt/trn_rl_repo")

import ml_dtypes

BF16 = ml_dtypes.bfloat16

# ---------------------------------------------------------------- real config
REAL_CFG = dict(
    N=50000, E=800000, IN=128, H=128, C=16, G=256, NC=8,
    HALF=32768, EPS=1e-5, GCH=6,
)


# ---------------------------------------------------------------- preprocess
def preprocess(inputs, cfg):
    """Host-side: shard + build all per-core data arrays and the static meta
    that shapes the program (identical across cores)."""
    N, NC, G, H = cfg["N"], cfg["NC"], cfg["G"], cfg["H"]
    IN, HALF = cfg["IN"], cfg["HALF"]
    NPC = N // NC
    NBLK = (NPC + 127) // 128
    LASTROWS = NPC - (NBLK - 1) * 128

    assert HALF <= 32768 and N - HALF <= 32768, "int16 gather index overflow"
    ei = np.asarray(inputs["edge_index"]).astype(np.int64)
    batch = np.asarray(inputs["batch"]).astype(np.int64)
    x = np.asarray(inputs["x"], np.float32)
    scalar = np.asarray(inputs["scalar"], np.float32)

    # deg counts self-loops (gcn_norm adds them), but self-loop messages are
    # applied on-device from the local transform (agg seeded with dinv^2 * u),
    # so they are excluded from the gathered edge lists.
    deg = (np.bincount(ei[1], minlength=N) + 1).astype(np.float32)
    dinv = 1.0 / np.sqrt(np.maximum(deg, 1.0))
    src_all = ei[0].copy()
    dst_all = ei[1].copy()

    # ---- relabel nodes within each core to balance per-block chunk counts.
    # Block capacity is 128 (last block LASTROWS); greedily pack nodes so the
    # max per-block lo/hi in-edge counts (-> CPB, the SPMD-uniform chunk
    # schedule) shrink toward the mean.
    lo_in = np.bincount(dst_all[src_all < HALF], minlength=N).astype(np.int64)
    hi_in = np.bincount(dst_all[src_all >= HALF], minlength=N).astype(np.int64)
    gp = np.empty(N, np.int64)  # old global id -> new global id
    for r in range(NC):
        nodes = np.arange(r * NPC, (r + 1) * NPC)
        order = np.argsort(-(lo_in[nodes] + hi_in[nodes]), kind="stable")
        cap = np.full(NBLK, 128, np.int64)
        cap[NBLK - 1] = LASTROWS
        lo_tot = float(lo_in[nodes].sum())
        hi_tot = float(hi_in[nodes].sum())
        lo_tgt = max(lo_tot / NBLK, 1.0)
        hi_tgt = max(hi_tot / NBLK, 1.0)
        # hard caps at the smallest feasible chunk count per flavor; fall
        # back to score-only choice if no block satisfies them.
        lo_cap = max(int(np.ceil(lo_tot / NBLK / 128)), 1) * 128
        hi_cap = max(int(np.ceil(hi_tot / NBLK / 128)), 1) * 128
        lo_load = np.zeros(NBLK, np.float64)
        hi_load = np.zeros(NBLK, np.float64)
        fill = np.zeros(NBLK, np.int64)
        for o in order:
            n = nodes[o]
            cand = np.nonzero(fill < cap)[0]
            ok = cand[(lo_load[cand] + lo_in[n] <= lo_cap)
                      & (hi_load[cand] + hi_in[n] <= hi_cap)]
            pick = ok if len(ok) else cand
            score = np.maximum((lo_load[pick] + lo_in[n]) / lo_tgt,
                               (hi_load[pick] + hi_in[n]) / hi_tgt)
            b = pick[np.argmin(score)]
            gp[n] = r * NPC + b * 128 + fill[b]
            lo_load[b] += lo_in[n]
            hi_load[b] += hi_in[n]
            fill[b] += 1
    # local ids within the last block exceed NPC bounds check:
    # new local id = b*128 + slot; for the last block slot < LASTROWS so
    # ids stay < NPC.  Apply the permutation to everything node-indexed.
    src_all = gp[src_all]
    dst_all = gp[dst_all]
    x2 = np.empty_like(x)
    x2[gp] = x
    x = x2
    batch2 = np.empty_like(batch)
    batch2[gp] = batch
    batch = batch2
    dinv2 = np.empty_like(dinv)
    dinv2[gp] = dinv
    dinv = dinv2

    # BN folding: layer l: h' = relu((agg + b - m) * k * g + be)
    #   A_l = k*g (folded into W on host), B_l = (b - m)*A_l + be
    gamma = np.asarray(inputs["gamma"], np.float32)
    beta = np.asarray(inputs["beta"], np.float32)
    mean = np.asarray(inputs["mean"], np.float32)
    var = np.asarray(inputs["var"], np.float32)
    W0 = np.asarray(inputs["W0"], np.float32)
    Wk = np.asarray(inputs["Wk"], np.float32)
    b0 = np.asarray(inputs["b0"], np.float32)
    bk = np.asarray(inputs["bk"], np.float32)
    Ws = np.asarray(inputs["Ws"], np.float32)
    bs = np.asarray(inputs["bs"], np.float32)
    Wc1 = np.asarray(inputs["Wc1"], np.float32)
    bc1 = np.asarray(inputs["bc1"], np.float32)
    Wc2 = np.asarray(inputs["Wc2"], np.float32)
    bc2 = np.asarray(inputs["bc2"], np.float32)

    Wl = [W0, Wk[0], Wk[1]]
    bl = [b0, bk[0], bk[1]]
    A = [gamma[i] / np.sqrt(var[i] + cfg["EPS"]) for i in range(3)]
    B = [(bl[i] - mean[i]) * A[i] + beta[i] for i in range(3)]
    Wp = [Wl[i] * A[i][None, :] for i in range(3)]  # W'_l

    # ---- per-core edge lists, chunked per (block, flavor) -----------------
    per_core = []
    cpb_lo_max = 0
    cpb_hi_max = 0
    for r in range(NC):
        m = (dst_all >= r * NPC) & (dst_all < (r + 1) * NPC)
        s = src_all[m]
        d = dst_all[m] - r * NPC
        blk = d // 128
        blocks = []
        for b in range(NBLK):
            mb = blk == b
            sb = s[mb]
            db = (d[mb] - b * 128).astype(np.int64)
            lo_m = sb < HALF
            pairs = []
            for flavor in (0, 1):
                mf = lo_m if flavor == 0 else ~lo_m
                sf = sb[mf] - (0 if flavor == 0 else HALF)
                df = db[mf]
                nch = int(np.ceil(len(sf) / 128)) if len(sf) else 0
                pairs.append((sf.astype(np.int64), df, nch))
            blocks.append(pairs)
            cpb_lo_max = max(cpb_lo_max, pairs[0][2])
            cpb_hi_max = max(cpb_hi_max, pairs[1][2])
        per_core.append(blocks)

    GCH = cfg["GCH"]
    CPB = (cpb_lo_max, cpb_hi_max)
    NLO = CPB[0] * NBLK
    NHI = CPB[1] * NBLK
    # pad each flavor's stream so gather calls are uniform GCH-chunk slabs
    NCALL_LO = -(-NLO // GCH) if NLO else 0
    NCALL_HI = -(-NHI // GCH) if NHI else 0
    NLO_P = NCALL_LO * GCH
    NCH = NLO_P + NCALL_HI * GCH

    # ---- build per-core tensors -------------------------------------------
    graphs_per_core = []
    in_maps = []
    for r in range(NC):
        idx_flat = np.zeros((NCH, 128), np.int16)
        segval = np.zeros((NCH, 128), np.float32)
        segdst = np.zeros((NCH, 128), np.int64)
        for b in range(NBLK):
            for flavor in (0, 1):
                sf, df, nch = per_core[r][b][flavor]
                base = (b * CPB[0]) if flavor == 0 else (NLO_P + b * CPB[1])
                for j in range(nch):
                    lo_e = j * 128
                    hi_e = min(lo_e + 128, len(sf))
                    n = hi_e - lo_e
                    c = base + j
                    idx_flat[c, :n] = sf[lo_e:hi_e].astype(np.int16)
                    segval[c, :n] = 1.0
                    segdst[c, :n] = df[lo_e:hi_e]
        # dst-local per edge, bf16 (dummy edges -> 200, matching no column of
        # the on-device iota compare). [128(edge partition), NCH]
        dstcol = np.where(segval.T > 0, segdst.T.astype(np.float32),
                          200.0).astype(BF16)
        # idx SBUF layout: element i of the stream lives at partition i%16,
        # free slot i//16; replicated x8 down the 128 partitions.
        idx16 = idx_flat.reshape(NCH, 8, 16).transpose(2, 0, 1).reshape(16, NCH * 8)
        idx128 = np.tile(idx16, (8, 1))

        # node-major -> feat-major transposed x slice, bf16, padded to NBLK*128
        xs = x[r * NPC:(r + 1) * NPC]
        xT = np.zeros((IN, NBLK * 128), np.float32)
        xT[:, :NPC] = xs.T
        dinv_r = np.zeros((NBLK * 128,), np.float32)
        dinv_r[:NPC] = dinv[r * NPC:(r + 1) * NPC]
        dinvrep = np.broadcast_to(dinv_r, (128, NBLK * 128)).astype(BF16)
        dinvcols = dinv_r.reshape(NBLK, 128).T.copy()  # [128, NBLK]

        # graph one-hot [128, NBLK, G] (partition-major for clean DMA)
        gm = np.zeros((128, NBLK, G), np.float32)
        bt = batch[r * NPC:(r + 1) * NPC]
        nn = np.arange(NPC)
        gm[nn % 128, nn // 128, bt] = 1.0
        graphs_per_core.append(len(np.unique(bt)))

        counts = np.bincount(batch, minlength=G).astype(np.float32)
        cinv = 1.0 / np.maximum(counts, 1.0)
        cinvrep = np.broadcast_to(cinv, (128, G)).copy()

        iota_big = np.broadcast_to(
            np.arange(128, dtype=np.float32),
            (128, GCH, 128)).astype(BF16).copy()
        in_map = dict(
            xT=xT.astype(BF16),
            dstcol=dstcol,
            iota=iota_big,
            idx=idx128.astype(np.int16),
            dinvrep=dinvrep,
            dinvcols=dinvcols,
            Bcols=np.stack(B, axis=1).astype(np.float32),          # [H, 3]
            W0p=Wp[0].astype(BF16),
            W1p=Wp[1].astype(BF16),
            W2p=Wp[2].astype(BF16),
            ident=np.eye(128, dtype=BF16),
            gm=gm.astype(BF16),
            cinvrep=cinvrep,
            scalarT=scalar.T.copy().astype(np.float32),            # [8, G]
            Ws=Ws,                                                 # [8, 64]
            bs_col=bs.reshape(-1, 1),
            Wc1a=Wc1[:H].copy(),
            Wc1b=Wc1[H:].copy(),
            bc1_col=bc1.reshape(-1, 1),
            Wc2=Wc2,
            bc2_col=bc2.reshape(-1, 1),
        )
        in_maps.append(in_map)

    meta = dict(
        cfg=cfg, NPC=NPC, NBLK=NBLK, LASTROWS=LASTROWS,
        CPB=CPB, NLO=NLO, NHI=NHI, NCH=NCH, NLO_P=NLO_P,
        NCALL_LO=NCALL_LO, NCALL_HI=NCALL_HI,
        shapes={k: (tuple(v.shape), v.dtype) for k, v in in_maps[0].items()},
    )
    assert max(graphs_per_core) <= G
    return meta, in_maps


# ---------------------------------------------------------------- program
def build_program(meta):
    import concourse.bass as bass
    import concourse.bacc as bacc
    import concourse.mybir as mybir
    import concourse.tile as tile

    cfg = meta["cfg"]
    N, NC, G, H, IN = cfg["N"], cfg["NC"], cfg["G"], cfg["H"], cfg["IN"]
    C, HALF, GCH = cfg["C"], cfg["HALF"], cfg["GCH"]
    NBLK, LASTROWS = meta["NBLK"], meta["LASTROWS"]
    CPB, NCH, NLO_P = meta["CPB"], meta["NCH"], meta["NLO_P"]
    NCALL_LO, NCALL_HI = meta["NCALL_LO"], meta["NCALL_HI"]
    NPC = meta["NPC"]
    f32, bf16, i16 = mybir.dt.float32, mybir.dt.bfloat16, mybir.dt.int16
    Alu = mybir.AluOpType
    Act = mybir.ActivationFunctionType

    nc = bacc.Bacc("TRN2", target_bir_lowering=False, debug=False,
                   enable_asserts=True, num_devices=NC,
                   num_swdge_queues=4)

    def dram_in(name):
        shape, dtype = meta["shapes"][name]
        return nc.dram_tensor(name, list(shape), mybir.dt.from_np(np.dtype(dtype)),
                              kind="ExternalInput").ap()

    ins = {k: dram_in(k) for k in meta["shapes"]}
    out_dram = nc.dram_tensor("out", [C, G], f32, kind="ExternalOutput").ap()

    rg = [list(range(NC))]

    # gather call plan: stream = [lo calls][hi calls], each exactly GCH chunks
    calls = [(0, ci * GCH) for ci in range(NCALL_LO)] + \
            [(1, NLO_P + ci * GCH) for ci in range(NCALL_HI)]
    call_of_chunk = {}
    for ci, (fl, start) in enumerate(calls):
        for k in range(GCH):
            call_of_chunk[start + k] = (ci, k)

    with tile.TileContext(nc) as tc:
        with (
            tc.tile_pool(name="const", bufs=1) as constp,
            tc.tile_pool(name="msgs", bufs=14) as msgp,
            tc.tile_pool(name="segs", bufs=14) as segp,
            tc.tile_pool(name="work", bufs=3) as workp,
            tc.tile_pool(name="rows", bufs=3) as rowp,
            tc.tile_pool(name="psum", bufs=2, space="PSUM") as psump,
            tc.tile_pool(name="psumT", bufs=2, space="PSUM") as psumTp,
            tc.tile_pool(name="psumU", bufs=2, space="PSUM") as psumUp,
            tc.tile_pool(name="psumP", bufs=1, space="PSUM") as psumPp,
            tc.tile_pool(name="dram", bufs=1, space="DRAM") as dramp,
        ):
            # ---------------- constants into SBUF
            idx_sb = constp.tile([128, NCH * 8], i16)
            nc.sync.dma_start(idx_sb[:], ins["idx"][:])
            dstcol_sb = constp.tile([128, NCH], bf16)
            nc.sync.dma_start(dstcol_sb[:], ins["dstcol"][:])
            iota_sb = constp.tile([128, GCH, 128], bf16)
            nc.sync.dma_start(iota_sb[:], ins["iota"][:])
            xT_sb = constp.tile([IN, NBLK * 128], bf16)
            nc.sync.dma_start(xT_sb[:], ins["xT"][:])
            dinvrep_sb = constp.tile([128, NBLK * 128], bf16)
            nc.sync.dma_start(dinvrep_sb[:], ins["dinvrep"][:])
            dinvcols_sb = constp.tile([128, NBLK], f32)
            nc.sync.dma_start(dinvcols_sb[:], ins["dinvcols"][:])
            Bcols_sb = constp.tile([H, 3], f32)
            nc.sync.dma_start(Bcols_sb[:], ins["Bcols"][:])
            W_sb = []
            for wname in ("W0p", "W1p", "W2p"):
                w = constp.tile([H, H], bf16, name=f"w_{wname}")
                nc.sync.dma_start(w[:], ins[wname][:])
                W_sb.append(w)
            ident_sb = constp.tile([128, 128], bf16)
            nc.sync.dma_start(ident_sb[:], ins["ident"][:])
            cinvrep_sb = constp.tile([128, G], f32)
            nc.sync.dma_start(cinvrep_sb[:], ins["cinvrep"][:])
            scalarT_sb = constp.tile([8, G], f32)
            nc.sync.dma_start(scalarT_sb[:], ins["scalarT"][:])
            Ws_sb = constp.tile([8, 64], f32)
            nc.sync.dma_start(Ws_sb[:], ins["Ws"][:])
            bs_sb = constp.tile([64, 1], f32)
            nc.sync.dma_start(bs_sb[:], ins["bs_col"][:])
            Wc1a_sb = constp.tile([H, H], f32)
            nc.sync.dma_start(Wc1a_sb[:], ins["Wc1a"][:])
            Wc1b_sb = constp.tile([64, H], f32)
            nc.sync.dma_start(Wc1b_sb[:], ins["Wc1b"][:])
            bc1_sb = constp.tile([H, 1], f32)
            nc.sync.dma_start(bc1_sb[:], ins["bc1_col"][:])
            Wc2_sb = constp.tile([H, C], f32)
            nc.sync.dma_start(Wc2_sb[:], ins["Wc2"][:])
            bc2_sb = constp.tile([C, 1], f32)
            nc.sync.dma_start(bc2_sb[:], ins["bc2_col"][:])

            # ping-pong store of the (pre-dinv) transformed features U^T per
            # block; layer l seeds its aggregation psum with
            # usave[l%2] * dinv_d (self-loop term) instead of gathering loops.
            usave = [constp.tile([H, NBLK * 128], bf16, name=f"usave{i}")
                     for i in range(2)]

            # ---------------- DRAM tables + collective buffers
            shared = "Shared" if NC > 4 else "Local"
            T = [dramp.tile([N, H], bf16, name=f"table{l}", addr_space=shared)
                 for l in range(3)]
            agin = [dramp.tile([NPC, H], bf16, name=f"agin{l}") for l in range(3)]
            ar_in = dramp.tile([128, G], bf16, name="ar_in")
            ar_out = dramp.tile([128, G], bf16, name="ar_out", addr_space=shared)

            def agin_write(l, b, rows, nr):
                nc.sync.dma_start(agin[l][b * 128: b * 128 + nr, :], rows[:nr, :])

            # ---------------- stage A: T0 = dinv * (x @ W0')
            for b in range(NBLK):
                u0 = psumUp.tile([H, 128], f32, tag="upsum")
                nc.tensor.matmul(u0[:], W_sb[0][:], xT_sb[:, b * 128:(b + 1) * 128],
                                 start=True, stop=True)
                u0s = usave[0][:, b * 128:(b + 1) * 128]
                nc.vector.tensor_copy(u0s, u0[:])
                tps = psumTp.tile([128, H], bf16, tag="tpsum")
                nc.tensor.transpose(tps[:], u0s, ident_sb[:])
                rows = rowp.tile([128, H], bf16, tag="rows")
                nc.scalar.activation(rows[:], tps[:], Act.Copy,
                                     scale=dinvcols_sb[:, b:b + 1])
                nr = 128 if b < NBLK - 1 else LASTROWS
                agin_write(0, b, rows, nr)

            # ---------------- layers
            for l in range(3):
                nc.gpsimd.collective_compute(
                    "AllGather", Alu.bypass, replica_groups=rg,
                    ins=[agin[l].opt()], outs=[T[l].opt()],
                )
                lo_view = T[l][0:HALF, :]
                hi_view = T[l][HALF:N, :]

                msg_tiles = {}
                seg_tiles = {}

                def ensure_call(ci):
                    if ci in msg_tiles:
                        return
                    fl, start = calls[ci]
                    mt = msgp.tile([128, GCH, H], bf16, tag=f"msg{fl}",
                                   name=f"msg_{l}_{ci}")
                    st = segp.tile([128, GCH, 128], bf16, tag=f"seg{fl}",
                                   name=f"seg_{l}_{ci}")
                    view = lo_view if fl == 0 else hi_view
                    nidx = GCH * 128
                    nc.gpsimd.dma_gather(
                        mt[:], view, idx_sb[:, start * 8:(start + GCH) * 8],
                        nidx, nidx, H, single_packet=(GCH * 128 <= 1024),
                        queue_num=ci % 4,
                    )
                    # one-hot seg built on-device: st[p,c,d] = (d == dstcol[p,c])
                    nc.vector.tensor_tensor(
                        st[:], iota_sb[:],
                        dstcol_sb[:, start:start + GCH][:, :, None]
                        .broadcast_to((128, GCH, 128)),
                        op=Alu.is_equal)
                    msg_tiles[ci] = mt
                    seg_tiles[ci] = st

                for b in range(NBLK):
                    agg = psump.tile([H, 128], f32, tag="agg", name=f"agg_{l}_{b}")
                    # seed with the self-loop term: (after the epilogue's
                    # * dinv_d this becomes dinv_d^2 * (h @ W')_d)
                    nc.vector.tensor_tensor(
                        agg[:], usave[l % 2][:, b * 128:(b + 1) * 128],
                        dinvrep_sb[:, b * 128:(b + 1) * 128], op=Alu.mult)
                    chunk_ids = (
                        [b * CPB[0] + j for j in range(CPB[0])]
                        + [NLO_P + b * CPB[1] + j for j in range(CPB[1])]
                    )
                    for c in chunk_ids:
                        ci, slot = call_of_chunk[c]
                        ensure_call(ci)
                        nc.tensor.matmul(
                            agg[:], msg_tiles[ci][:, slot, :],
                            seg_tiles[ci][:, slot, :],
                            start=False, stop=False, skip_group_check=True,
                        )
                    # epilogue: t = agg * dinv_d ; h^T = relu(t + B_l)
                    tmp = workp.tile([H, 128], f32, tag="tmp")
                    nc.vector.tensor_tensor(
                        tmp[:], agg[:], dinvrep_sb[:, b * 128:(b + 1) * 128],
                        op=Alu.mult)
                    hT = workp.tile([H, 128], bf16, tag="hT")
                    nc.scalar.activation(hT[:], tmp[:], Act.Relu,
                                         bias=Bcols_sb[:, l:l + 1])
                    if l < 2:
                        # U^T = W'_{l+1}.T @ h^T ; then * dinv_n, transpose out
                        ups = psumUp.tile([H, 128], f32, tag="upsum")
                        nc.tensor.matmul(ups[:], W_sb[l + 1][:], hT[:],
                                         start=True, stop=True)
                        us = usave[(l + 1) % 2][:, b * 128:(b + 1) * 128]
                        nc.vector.tensor_copy(us, ups[:])
                        tps = psumTp.tile([128, H], bf16, tag="tpsum")
                        nc.tensor.transpose(tps[:], us, ident_sb[:])
                        rows = rowp.tile([128, H], bf16, tag="rows")
                        nc.scalar.activation(rows[:], tps[:], Act.Copy,
                                             scale=dinvcols_sb[:, b:b + 1])
                        nr = 128 if b < NBLK - 1 else LASTROWS
                        agin_write(l + 1, b, rows, nr)
                    else:
                        # pool: h3 node-major, then pooled^T += h3.T' ...
                        tps = psumTp.tile([128, H], bf16, tag="tpsum")
                        nc.tensor.transpose(tps[:], hT[:], ident_sb[:])
                        h3 = workp.tile([128, H], bf16, tag="h3")
                        nc.vector.tensor_copy(h3[:], tps[:])
                        gm_t = rowp.tile([128, G], bf16, tag="gm")
                        nc.sync.dma_start(gm_t[:], ins["gm"][:, b, :])
                        if b == 0:
                            pool_ps = psumPp.tile([H, G], f32, name="pool_ps")
                        nc.tensor.matmul(pool_ps[:], h3[:], gm_t[:],
                                         start=(b == 0), stop=(b == NBLK - 1),
                                         skip_group_check=True)

            # ---------------- pooled partial -> AllReduce (bf16 wire)
            pool_sb = workp.tile([H, G], bf16, name="pool_sb")
            nc.vector.tensor_copy(pool_sb[:], pool_ps[:])
            nc.sync.dma_start(ar_in[:], pool_sb[:])
            nc.gpsimd.collective_compute(
                "AllReduce", Alu.add, replica_groups=rg,
                ins=[ar_in.opt()], outs=[ar_out.opt()],
            )
            gsum = workp.tile([H, G], bf16, name="gsum")
            nc.sync.dma_start(gsum[:], ar_out[:])
            gembT = workp.tile([H, G], f32, name="gembT")
            nc.vector.tensor_tensor(gembT[:], gsum[:], cinvrep_sb[:], op=Alu.mult)

            # ---------------- head (fp32, replicated on every core)
            semb_ps = psumUp.tile([64, G], f32, tag="upsum", name="semb_ps")
            nc.tensor.matmul(semb_ps[:], Ws_sb[:], scalarT_sb[:],
                             start=True, stop=True)
            sembT = workp.tile([64, G], f32, name="sembT")
            nc.scalar.activation(sembT[:], semb_ps[:], Act.Relu, bias=bs_sb[:])

            z_ps = psumPp.tile([H, G], f32, name="z_ps")
            nc.tensor.matmul(z_ps[:], Wc1a_sb[:], gembT[:],
                             start=True, stop=False, skip_group_check=True)
            nc.tensor.matmul(z_ps[:], Wc1b_sb[:], sembT[:],
                             start=False, stop=True, skip_group_check=True)
            z2T = workp.tile([H, G], f32, name="z2T")
            nc.scalar.activation(z2T[:], z_ps[:], Act.Relu, bias=bc1_sb[:])

            o_ps = psumUp.tile([C, G], f32, tag="upsum", name="o_ps")
            nc.tensor.matmul(o_ps[:], Wc2_sb[:], z2T[:], start=True, stop=True)
            o_sb = workp.tile([C, G], f32, name="o_sb")
            nc.scalar.activation(o_sb[:], o_ps[:], Act.Identity, bias=bc2_sb[:])
            nc.sync.dma_start(out_dram[:], o_sb[:])

    nc.compile()
    return nc


# ---------------------------------------------------------------- runner
_CACHE = {}


def run(inputs, cfg=None, trace=False):
    cfg = cfg or REAL_CFG
    meta, in_maps = preprocess(inputs, cfg)
    key = (tuple(sorted(cfg.items())), meta["CPB"], meta["NCH"])
    if key not in _CACHE:
        _CACHE[key] = build_program(meta)
    nc = _CACHE[key]
    from concourse import bass_utils
    res = bass_utils.run_bass_kernel_spmd(
        nc, in_maps, core_ids=list(range(cfg["NC"])), trace=trace,
    )
    out = np.asarray(res.results[0]["out"], np.float32).T.copy()
    return out, res


def kernel(**inputs) -> np.ndarray:
    out, _ = run(inputs, REAL_CFG, trace=False)
    return out



# revision 45
# speedup vs baseline: 1.0628x; 1.0628x over previous
"""Trainium2 Bass kernel for BracketGNN (3-layer GCN + mean-pool + MLP head).

Strategy (8 NeuronCores, SPMD):
  - Nodes sharded contiguously across cores (6250 dst nodes / core); each core
    owns the incident edges of its dst nodes.
  - Per layer the full node table T_l[n] = dinv_n * (h_l @ W'_l)  (bf16,
    [N,128] rows in DRAM) is AllGathered; each core edge-gathers rows of its
    in-edges with the MoE dma_gather primitive (int16 indices -> the table is
    addressed through two base views, src<32768 and src>=32768).
  - The gathers are issued round-robin over the 4 SWDGE queues so descriptor
    generation runs on all four Q7 core pairs concurrently (the dominant cost
    of this kernel is SWDGE descriptor generation at ~7.4 ns/row/pair); small
    GCH (6 chunks/call) + deep msg/seg pools keep all four queues fed.
  - Aggregation = per-128-edge-chunk matmul:  psum[feat, dst] +=
    msg_chunk[128e x 128f].T @ seg_chunk[128e x 128dst]  where seg is the
    one-hot (edge -> local dst) matrix built on-device by DVE compare against
    a host-provided dst-column table.  Chunk schedule is uniform across cores
    (SPMD); per-core variation lives in the seg/idx data.
  - Self-loop messages never go through the gather: the aggregation psum is
    seeded with usave * dinv_d (usave = the block's own pre-dinv transform,
    saved from the previous layer's epilogue), which after the epilogue's
    * dinv_d equals the dinv_d^2 * (h @ W') self-loop term.
  - Epilogue per 128-dst block: x dinv_d, +B (folded BN bias), ReLU, next-layer
    weight transform (BN scale folded into W on host), x dinv_n, PE-transpose
    back to node-major rows, DMA into the next AllGather input.
  - After layer 3: per-graph mean pool via one-hot graph matmul (one-hot
    streamed from DRAM), bf16 AllReduce of the [128f x 256g] partial sums,
    then the (replicated) fp32 classifier head.
"""

import os
import sys
import numpy as np

sys.path.insert(0, "/opt/trn_rl_repo")

import ml_dtypes

BF16 = ml_dtypes.bfloat16

# ---------------------------------------------------------------- real config
REAL_CFG = dict(
    N=50000, E=800000, IN=128, H=128, C=16, G=256, NC=8,
    HALF=32768, EPS=1e-5, GCH=6,
)


# ---------------------------------------------------------------- preprocess
def preprocess(inputs, cfg):
    """Host-side: shard + build all per-core data arrays and the static meta
    that shapes the program (identical across cores)."""
    N, NC, G, H = cfg["N"], cfg["NC"], cfg["G"], cfg["H"]
    IN, HALF = cfg["IN"], cfg["HALF"]
    NPC = N // NC
    NBLK = (NPC + 127) // 128
    LASTROWS = NPC - (NBLK - 1) * 128

    assert HALF <= 32768 and N - HALF <= 32768, "int16 gather index overflow"
    ei = np.asarray(inputs["edge_index"]).astype(np.int64)
    batch = np.asarray(inputs["batch"]).astype(np.int64)
    x = np.asarray(inputs["x"], np.float32)
    scalar = np.asarray(inputs["scalar"], np.float32)

    # deg counts self-loops (gcn_norm adds them), but self-loop messages are
    # applied on-device from the local transform (agg seeded with dinv^2 * u),
    # so they are excluded from the gathered edge lists.
    deg = (np.bincount(ei[1], minlength=N) + 1).astype(np.float32)
    dinv = 1.0 / np.sqrt(np.maximum(deg, 1.0))
    src_all = ei[0].copy()
    dst_all = ei[1].copy()

    # ---- relabel nodes within each core to balance per-block chunk counts.
    # Block capacity is 128 (last block LASTROWS); greedily pack nodes so the
    # max per-block lo/hi in-edge counts (-> CPB, the SPMD-uniform chunk
    # schedule) shrink toward the mean.
    lo_in = np.bincount(dst_all[src_all < HALF], minlength=N).astype(np.int64)
    hi_in = np.bincount(dst_all[src_all >= HALF], minlength=N).astype(np.int64)
    gp = np.empty(N, np.int64)  # old global id -> new global id
    for r in range(NC):
        nodes = np.arange(r * NPC, (r + 1) * NPC)
        order = np.argsort(-(lo_in[nodes] + hi_in[nodes]), kind="stable")
        cap = np.full(NBLK, 128, np.int64)
        cap[NBLK - 1] = LASTROWS
        lo_tot = float(lo_in[nodes].sum())
        hi_tot = float(hi_in[nodes].sum())
        lo_tgt = max(lo_tot / NBLK, 1.0)
        hi_tgt = max(hi_tot / NBLK, 1.0)
        # hard caps at the smallest feasible chunk count per flavor; fall
        # back to score-only choice if no block satisfies them.
        lo_cap = max(int(np.ceil(lo_tot / NBLK / 128)), 1) * 128
        hi_cap = max(int(np.ceil(hi_tot / NBLK / 128)), 1) * 128
        lo_load = np.zeros(NBLK, np.float64)
        hi_load = np.zeros(NBLK, np.float64)
        fill = np.zeros(NBLK, np.int64)
        for o in order:
            n = nodes[o]
            cand = np.nonzero(fill < cap)[0]
            ok = cand[(lo_load[cand] + lo_in[n] <= lo_cap)
                      & (hi_load[cand] + hi_in[n] <= hi_cap)]
            pick = ok if len(ok) else cand
            score = np.maximum((lo_load[pick] + lo_in[n]) / lo_tgt,
                               (hi_load[pick] + hi_in[n]) / hi_tgt)
            b = pick[np.argmin(score)]
            gp[n] = r * NPC + b * 128 + fill[b]
            lo_load[b] += lo_in[n]
            hi_load[b] += hi_in[n]
            fill[b] += 1
    # local ids within the last block exceed NPC bounds check:
    # new local id = b*128 + slot; for the last block slot < LASTROWS so
    # ids stay < NPC.  Apply the permutation to everything node-indexed.
    src_all = gp[src_all]
    dst_all = gp[dst_all]
    x2 = np.empty_like(x)
    x2[gp] = x
    x = x2
    batch2 = np.empty_like(batch)
    batch2[gp] = batch
    batch = batch2
    dinv2 = np.empty_like(dinv)
    dinv2[gp] = dinv
    dinv = dinv2

    # BN folding: layer l: h' = relu((agg + b - m) * k * g + be)
    #   A_l = k*g (folded into W on host), B_l = (b - m)*A_l + be
    gamma = np.asarray(inputs["gamma"], np.float32)
    beta = np.asarray(inputs["beta"], np.float32)
    mean = np.asarray(inputs["mean"], np.float32)
    var = np.asarray(inputs["var"], np.float32)
    W0 = np.asarray(inputs["W0"], np.float32)
    Wk = np.asarray(inputs["Wk"], np.float32)
    b0 = np.asarray(inputs["b0"], np.float32)
    bk = np.asarray(inputs["bk"], np.float32)
    Ws = np.asarray(inputs["Ws"], np.float32)
    bs = np.asarray(inputs["bs"], np.float32)
    Wc1 = np.asarray(inputs["Wc1"], np.float32)
    bc1 = np.asarray(inputs["bc1"], np.float32)
    Wc2 = np.asarray(inputs["Wc2"], np.float32)
    bc2 = np.asarray(inputs["bc2"], np.float32)

    Wl = [W0, Wk[0], Wk[1]]
    bl = [b0, bk[0], bk[1]]
    A = [gamma[i] / np.sqrt(var[i] + cfg["EPS"]) for i in range(3)]
    B = [(bl[i] - mean[i]) * A[i] + beta[i] for i in range(3)]
    Wp = [Wl[i] * A[i][None, :] for i in range(3)]  # W'_l

    # ---- per-core edge lists, chunked per (block, flavor) -----------------
    per_core = []
    cpb_lo_max = 0
    cpb_hi_max = 0
    for r in range(NC):
        m = (dst_all >= r * NPC) & (dst_all < (r + 1) * NPC)
        s = src_all[m]
        d = dst_all[m] - r * NPC
        blk = d // 128
        blocks = []
        for b in range(NBLK):
            mb = blk == b
            sb = s[mb]
            db = (d[mb] - b * 128).astype(np.int64)
            lo_m = sb < HALF
            pairs = []
            for flavor in (0, 1):
                mf = lo_m if flavor == 0 else ~lo_m
                sf = sb[mf] - (0 if flavor == 0 else HALF)
                df = db[mf]
                nch = int(np.ceil(len(sf) / 128)) if len(sf) else 0
                pairs.append((sf.astype(np.int64), df, nch))
            blocks.append(pairs)
            cpb_lo_max = max(cpb_lo_max, pairs[0][2])
            cpb_hi_max = max(cpb_hi_max, pairs[1][2])
        per_core.append(blocks)

    GCH = cfg["GCH"]
    CPB = (cpb_lo_max, cpb_hi_max)
    NLO = CPB[0] * NBLK
    NHI = CPB[1] * NBLK
    # pad each flavor's stream so gather calls are uniform GCH-chunk slabs
    NCALL_LO = -(-NLO // GCH) if NLO else 0
    NCALL_HI = -(-NHI // GCH) if NHI else 0
    NLO_P = NCALL_LO * GCH
    NCH = NLO_P + NCALL_HI * GCH

    # ---- build per-core tensors -------------------------------------------
    graphs_per_core = []
    in_maps = []
    for r in range(NC):
        idx_flat = np.zeros((NCH, 128), np.int16)
        segval = np.zeros((NCH, 128), np.float32)
        segdst = np.zeros((NCH, 128), np.int64)
        for b in range(NBLK):
            for flavor in (0, 1):
                sf, df, nch = per_core[r][b][flavor]
                base = (b * CPB[0]) if flavor == 0 else (NLO_P + b * CPB[1])
                for j in range(nch):
                    lo_e = j * 128
                    hi_e = min(lo_e + 128, len(sf))
                    n = hi_e - lo_e
                    c = base + j
                    idx_flat[c, :n] = sf[lo_e:hi_e].astype(np.int16)
                    segval[c, :n] = 1.0
                    segdst[c, :n] = df[lo_e:hi_e]
        # dst-local per edge, bf16 (dummy edges -> 200, matching no column of
        # the on-device iota compare). [128(edge partition), NCH]
        dstcol = np.where(segval.T > 0, segdst.T.astype(np.float32),
                          200.0).astype(BF16)
        # idx SBUF layout: element i of the stream lives at partition i%16,
        # free slot i//16; replicated x8 down the 128 partitions.
        idx16 = idx_flat.reshape(NCH, 8, 16).transpose(2, 0, 1).reshape(16, NCH * 8)
        idx128 = np.tile(idx16, (8, 1))

        # node-major -> feat-major transposed x slice, bf16, padded to NBLK*128
        xs = x[r * NPC:(r + 1) * NPC]
        xT = np.zeros((IN, NBLK * 128), np.float32)
        xT[:, :NPC] = xs.T
        dinv_r = np.zeros((NBLK * 128,), np.float32)
        dinv_r[:NPC] = dinv[r * NPC:(r + 1) * NPC]
        dinvrep = np.broadcast_to(dinv_r, (128, NBLK * 128)).astype(BF16)
        dinvcols = dinv_r.reshape(NBLK, 128).T.copy()  # [128, NBLK]

        # graph one-hot [128, NBLK, G] (partition-major for clean DMA)
        gm = np.zeros((128, NBLK, G), np.float32)
        bt = batch[r * NPC:(r + 1) * NPC]
        nn = np.arange(NPC)
        gm[nn % 128, nn // 128, bt] = 1.0
        graphs_per_core.append(len(np.unique(bt)))

        counts = np.bincount(batch, minlength=G).astype(np.float32)
        cinv = 1.0 / np.maximum(counts, 1.0)
        cinvrep = np.broadcast_to(cinv, (128, G)).copy()

        iota_big = np.broadcast_to(
            np.arange(128, dtype=np.float32),
            (128, GCH, 128)).astype(BF16).copy()
        in_map = dict(
            xT=xT.astype(BF16),
            dstcol=dstcol,
            iota=iota_big,
            idx=idx128.astype(np.int16),
            dinvrep=dinvrep,
            dinvcols=dinvcols,
            Bcols=np.stack(B, axis=1).astype(np.float32),          # [H, 3]
            W0p=Wp[0].astype(BF16),
            W1p=Wp[1].astype(BF16),
            W2p=Wp[2].astype(BF16),
            ident=np.eye(128, dtype=BF16),
            gm=gm.astype(BF16),
            cinvrep=cinvrep,
            scalarT=scalar.T.copy().astype(np.float32),            # [8, G]
            Ws=Ws,                                                 # [8, 64]
            bs_col=bs.reshape(-1, 1),
            Wc1a=Wc1[:H].copy(),
            Wc1b=Wc1[H:].copy(),
            bc1_col=bc1.reshape(-1, 1),
            Wc2=Wc2,
            bc2_col=bc2.reshape(-1, 1),
        )
        in_maps.append(in_map)

    meta = dict(
        cfg=cfg, NPC=NPC, NBLK=NBLK, LASTROWS=LASTROWS,
        CPB=CPB, NLO=NLO, NHI=NHI, NCH=NCH, NLO_P=NLO_P,
        NCALL_LO=NCALL_LO, NCALL_HI=NCALL_HI,
        shapes={k: (tuple(v.shape), v.dtype) for k, v in in_maps[0].items()},
    )
    assert max(graphs_per_core) <= G
    return meta, in_maps


# ---------------------------------------------------------------- program
def build_program(meta):
    import concourse.bass as bass
    import concourse.bacc as bacc
    import concourse.mybir as mybir
    import concourse.tile as tile

    cfg = meta["cfg"]
    N, NC, G, H, IN = cfg["N"], cfg["NC"], cfg["G"], cfg["H"], cfg["IN"]
    C, HALF, GCH = cfg["C"], cfg["HALF"], cfg["GCH"]
    NBLK, LASTROWS = meta["NBLK"], meta["LASTROWS"]
    CPB, NCH, NLO_P = meta["CPB"], meta["NCH"], meta["NLO_P"]
    NCALL_LO, NCALL_HI = meta["NCALL_LO"], meta["NCALL_HI"]
    NPC = meta["NPC"]
    f32, bf16, i16 = mybir.dt.float32, mybir.dt.bfloat16, mybir.dt.int16
    Alu = mybir.AluOpType
    Act = mybir.ActivationFunctionType

    nc = bacc.Bacc("TRN2", target_bir_lowering=False, debug=False,
                   enable_asserts=True, num_devices=NC,
                   num_swdge_queues=4)

    def dram_in(name):
        shape, dtype = meta["shapes"][name]
        return nc.dram_tensor(name, list(shape), mybir.dt.from_np(np.dtype(dtype)),
                              kind="ExternalInput").ap()

    ins = {k: dram_in(k) for k in meta["shapes"]}
    out_dram = nc.dram_tensor("out", [C, G], f32, kind="ExternalOutput").ap()

    rg = [list(range(NC))]

    # gather call plan: stream = [lo calls][hi calls], each exactly GCH chunks
    calls = [(0, ci * GCH) for ci in range(NCALL_LO)] + \
            [(1, NLO_P + ci * GCH) for ci in range(NCALL_HI)]
    call_of_chunk = {}
    for ci, (fl, start) in enumerate(calls):
        for k in range(GCH):
            call_of_chunk[start + k] = (ci, k)

    with tile.TileContext(nc) as tc:
        with (
            tc.tile_pool(name="const", bufs=1) as constp,
            tc.tile_pool(name="msgs", bufs=16) as msgp,
            tc.tile_pool(name="segs", bufs=16) as segp,
            tc.tile_pool(name="work", bufs=3) as workp,
            tc.tile_pool(name="rows", bufs=3) as rowp,
            tc.tile_pool(name="psum", bufs=2, space="PSUM") as psump,
            tc.tile_pool(name="psumT", bufs=2, space="PSUM") as psumTp,
            tc.tile_pool(name="psumU", bufs=2, space="PSUM") as psumUp,
            tc.tile_pool(name="psumP", bufs=1, space="PSUM") as psumPp,
            tc.tile_pool(name="dram", bufs=1, space="DRAM") as dramp,
        ):
            # ---------------- constants into SBUF
            idx_sb = constp.tile([128, NCH * 8], i16)
            nc.sync.dma_start(idx_sb[:], ins["idx"][:])
            dstcol_sb = constp.tile([128, NCH], bf16)
            nc.sync.dma_start(dstcol_sb[:], ins["dstcol"][:])
            iota_sb = constp.tile([128, GCH, 128], bf16)
            nc.sync.dma_start(iota_sb[:], ins["iota"][:])
            xT_sb = constp.tile([IN, NBLK * 128], bf16)
            nc.sync.dma_start(xT_sb[:], ins["xT"][:])
            dinvrep_sb = constp.tile([128, NBLK * 128], bf16)
            nc.sync.dma_start(dinvrep_sb[:], ins["dinvrep"][:])
            dinvcols_sb = constp.tile([128, NBLK], f32)
            nc.sync.dma_start(dinvcols_sb[:], ins["dinvcols"][:])
            Bcols_sb = constp.tile([H, 3], f32)
            nc.sync.dma_start(Bcols_sb[:], ins["Bcols"][:])
            W_sb = []
            for wname in ("W0p", "W1p", "W2p"):
                w = constp.tile([H, H], bf16, name=f"w_{wname}")
                nc.sync.dma_start(w[:], ins[wname][:])
                W_sb.append(w)
            ident_sb = constp.tile([128, 128], bf16)
            nc.sync.dma_start(ident_sb[:], ins["ident"][:])
            cinvrep_sb = constp.tile([128, G], f32)
            nc.sync.dma_start(cinvrep_sb[:], ins["cinvrep"][:])
            scalarT_sb = constp.tile([8, G], f32)
            nc.sync.dma_start(scalarT_sb[:], ins["scalarT"][:])
            Ws_sb = constp.tile([8, 64], f32)
            nc.sync.dma_start(Ws_sb[:], ins["Ws"][:])
            bs_sb = constp.tile([64, 1], f32)
            nc.sync.dma_start(bs_sb[:], ins["bs_col"][:])
            Wc1a_sb = constp.tile([H, H], f32)
            nc.sync.dma_start(Wc1a_sb[:], ins["Wc1a"][:])
            Wc1b_sb = constp.tile([64, H], f32)
            nc.sync.dma_start(Wc1b_sb[:], ins["Wc1b"][:])
            bc1_sb = constp.tile([H, 1], f32)
            nc.sync.dma_start(bc1_sb[:], ins["bc1_col"][:])
            Wc2_sb = constp.tile([H, C], f32)
            nc.sync.dma_start(Wc2_sb[:], ins["Wc2"][:])
            bc2_sb = constp.tile([C, 1], f32)
            nc.sync.dma_start(bc2_sb[:], ins["bc2_col"][:])

            # ping-pong store of the (pre-dinv) transformed features U^T per
            # block; layer l seeds its aggregation psum with
            # usave[l%2] * dinv_d (self-loop term) instead of gathering loops.
            usave = [constp.tile([H, NBLK * 128], bf16, name=f"usave{i}")
                     for i in range(2)]

            # ---------------- DRAM tables + collective buffers
            shared = "Shared" if NC > 4 else "Local"
            T = [dramp.tile([N, H], bf16, name=f"table{l}", addr_space=shared)
                 for l in range(3)]
            agin = [dramp.tile([NPC, H], bf16, name=f"agin{l}") for l in range(3)]
            ar_in = dramp.tile([128, G], bf16, name="ar_in")
            ar_out = dramp.tile([128, G], bf16, name="ar_out", addr_space=shared)

            def agin_write(l, b, rows, nr):
                nc.sync.dma_start(agin[l][b * 128: b * 128 + nr, :], rows[:nr, :])

            # ---------------- stage A: T0 = dinv * (x @ W0')
            for b in range(NBLK):
                u0 = psumUp.tile([H, 128], f32, tag="upsum")
                nc.tensor.matmul(u0[:], W_sb[0][:], xT_sb[:, b * 128:(b + 1) * 128],
                                 start=True, stop=True)
                u0s = usave[0][:, b * 128:(b + 1) * 128]
                nc.vector.tensor_copy(u0s, u0[:])
                tps = psumTp.tile([128, H], bf16, tag="tpsum")
                nc.tensor.transpose(tps[:], u0s, ident_sb[:])
                rows = rowp.tile([128, H], bf16, tag="rows")
                nc.scalar.activation(rows[:], tps[:], Act.Copy,
                                     scale=dinvcols_sb[:, b:b + 1])
                nr = 128 if b < NBLK - 1 else LASTROWS
                agin_write(0, b, rows, nr)

            # ---------------- layers
            for l in range(3):
                nc.gpsimd.collective_compute(
                    "AllGather", Alu.bypass, replica_groups=rg,
                    ins=[agin[l].opt()], outs=[T[l].opt()],
                )
                lo_view = T[l][0:HALF, :]
                hi_view = T[l][HALF:N, :]

                msg_tiles = {}
                seg_tiles = {}

                def ensure_call(ci):
                    if ci in msg_tiles:
                        return
                    fl, start = calls[ci]
                    mt = msgp.tile([128, GCH, H], bf16, tag=f"msg{fl}",
                                   name=f"msg_{l}_{ci}")
                    st = segp.tile([128, GCH, 128], bf16, tag=f"seg{fl}",
                                   name=f"seg_{l}_{ci}")
                    view = lo_view if fl == 0 else hi_view
                    nidx = GCH * 128
                    nc.gpsimd.dma_gather(
                        mt[:], view, idx_sb[:, start * 8:(start + GCH) * 8],
                        nidx, nidx, H, single_packet=(GCH * 128 <= 1024),
                        queue_num=ci % 4,
                    )
                    # one-hot seg built on-device: st[p,c,d] = (d == dstcol[p,c])
                    nc.vector.tensor_tensor(
                        st[:], iota_sb[:],
                        dstcol_sb[:, start:start + GCH][:, :, None]
                        .broadcast_to((128, GCH, 128)),
                        op=Alu.is_equal)
                    msg_tiles[ci] = mt
                    seg_tiles[ci] = st

                for b in range(NBLK):
                    agg = psump.tile([H, 128], f32, tag="agg", name=f"agg_{l}_{b}")
                    # seed with the self-loop term: (after the epilogue's
                    # * dinv_d this becomes dinv_d^2 * (h @ W')_d)
                    nc.vector.tensor_tensor(
                        agg[:], usave[l % 2][:, b * 128:(b + 1) * 128],
                        dinvrep_sb[:, b * 128:(b + 1) * 128], op=Alu.mult)
                    chunk_ids = (
                        [b * CPB[0] + j for j in range(CPB[0])]
                        + [NLO_P + b * CPB[1] + j for j in range(CPB[1])]
                    )
                    for c in chunk_ids:
                        ci, slot = call_of_chunk[c]
                        ensure_call(ci)
                        nc.tensor.matmul(
                            agg[:], msg_tiles[ci][:, slot, :],
                            seg_tiles[ci][:, slot, :],
                            start=False, stop=False, skip_group_check=True,
                        )
                    # epilogue: t = agg * dinv_d ; h^T = relu(t + B_l)
                    tmp = workp.tile([H, 128], f32, tag="tmp")
                    nc.vector.tensor_tensor(
                        tmp[:], agg[:], dinvrep_sb[:, b * 128:(b + 1) * 128],
                        op=Alu.mult)
                    hT = workp.tile([H, 128], bf16, tag="hT")
                    nc.scalar.activation(hT[:], tmp[:], Act.Relu,
                                         bias=Bcols_sb[:, l:l + 1])
                    if l < 2:
                        # U^T = W'_{l+1}.T @ h^T ; then * dinv_n, transpose out
                        ups = psumUp.tile([H, 128], f32, tag="upsum")
                        nc.tensor.matmul(ups[:], W_sb[l + 1][:], hT[:],
                                         start=True, stop=True)
                        us = usave[(l + 1) % 2][:, b * 128:(b + 1) * 128]
                        nc.vector.tensor_copy(us, ups[:])
                        tps = psumTp.tile([128, H], bf16, tag="tpsum")
                        nc.tensor.transpose(tps[:], us, ident_sb[:])
                        rows = rowp.tile([128, H], bf16, tag="rows")
                        nc.scalar.activation(rows[:], tps[:], Act.Copy,
                                             scale=dinvcols_sb[:, b:b + 1])
                        nr = 128 if b < NBLK - 1 else LASTROWS
                        agin_write(l + 1, b, rows, nr)
                    else:
                        # pool: h3 node-major, then pooled^T += h3.T' ...
                        tps = psumTp.tile([128, H], bf16, tag="tpsum")
                        nc.tensor.transpose(tps[:], hT[:], ident_sb[:])
                        h3 = workp.tile([128, H], bf16, tag="h3")
                        nc.vector.tensor_copy(h3[:], tps[:])
                        gm_t = rowp.tile([128, G], bf16, tag="gm")
                        nc.sync.dma_start(gm_t[:], ins["gm"][:, b, :])
                        if b == 0:
                            pool_ps = psumPp.tile([H, G], f32, name="pool_ps")
                        nc.tensor.matmul(pool_ps[:], h3[:], gm_t[:],
                                         start=(b == 0), stop=(b == NBLK - 1),
                                         skip_group_check=True)

            # ---------------- pooled partial -> AllReduce (bf16 wire)
            pool_sb = workp.tile([H, G], bf16, name="pool_sb")
            nc.vector.tensor_copy(pool_sb[:], pool_ps[:])
            nc.sync.dma_start(ar_in[:], pool_sb[:])
            nc.gpsimd.collective_compute(
                "AllReduce", Alu.add, replica_groups=rg,
                ins=[ar_in.opt()], outs=[ar_out.opt()],
            )
            gsum = workp.tile([H, G], bf16, name="gsum")
            nc.sync.dma_start(gsum[:], ar_out[:])
            gembT = workp.tile([H, G], f32, name="gembT")
            nc.vector.tensor_tensor(gembT[:], gsum[:], cinvrep_sb[:], op=Alu.mult)

            # ---------------- head (fp32, replicated on every core)
            semb_ps = psumUp.tile([64, G], f32, tag="upsum", name="semb_ps")
            nc.tensor.matmul(semb_ps[:], Ws_sb[:], scalarT_sb[:],
                             start=True, stop=True)
            sembT = workp.tile([64, G], f32, name="sembT")
            nc.scalar.activation(sembT[:], semb_ps[:], Act.Relu, bias=bs_sb[:])

            z_ps = psumPp.tile([H, G], f32, name="z_ps")
            nc.tensor.matmul(z_ps[:], Wc1a_sb[:], gembT[:],
                             start=True, stop=False, skip_group_check=True)
            nc.tensor.matmul(z_ps[:], Wc1b_sb[:], sembT[:],
                             start=False, stop=True, skip_group_check=True)
            z2T = workp.tile([H, G], f32, name="z2T")
            nc.scalar.activation(z2T[:], z_ps[:], Act.Relu, bias=bc1_sb[:])

            o_ps = psumUp.tile([C, G], f32, tag="upsum", name="o_ps")
            nc.tensor.matmul(o_ps[:], Wc2_sb[:], z2T[:], start=True, stop=True)
            o_sb = workp.tile([C, G], f32, name="o_sb")
            nc.scalar.activation(o_sb[:], o_ps[:], Act.Identity, bias=bc2_sb[:])
            nc.sync.dma_start(out_dram[:], o_sb[:])

    nc.compile()
    return nc


# ---------------------------------------------------------------- runner
_CACHE = {}


def run(inputs, cfg=None, trace=False):
    cfg = cfg or REAL_CFG
    meta, in_maps = preprocess(inputs, cfg)
    key = (tuple(sorted(cfg.items())), meta["CPB"], meta["NCH"])
    if key not in _CACHE:
        _CACHE[key] = build_program(meta)
    nc = _CACHE[key]
    from concourse import bass_utils
    res = bass_utils.run_bass_kernel_spmd(
        nc, in_maps, core_ids=list(range(cfg["NC"])), trace=trace,
    )
    out = np.asarray(res.results[0]["out"], np.float32).T.copy()
    return out, res


def kernel(**inputs) -> np.ndarray:
    out, _ = run(inputs, REAL_CFG, trace=False)
    return out

